# revision 21
# baseline (speedup 1.0000x reference)
"""Trainium2 Bass kernel for nn_Attention_45724221833663 (sparse_attention).

Strategy: data-parallel over batch B=8 across the 8 NeuronCores (one batch
element per core). All matmuls run in bf16 with fp32 PSUM accumulation
(fp8 was evaluated and rejected: softmax-weighted averaging preserves the
~3.6% fp8e4m3 per-element relative error, blowing the 2e-2 budget).

Per-core dataflow (all layouts chosen to avoid on-chip transposes of large
activations; weights and x are transposed on the host while sharding, and
ln_g/ln_b are folded into Wp/bp on the host):
  xcatT  [c=1024, kvp=1152]  (= concat(x_text, x).T, zero-padded 1101->1152)
  vw     [kvp, h, 65] = (xcatT.T @ WvT) interleaved per head + ones column
  qT     [o, n]    = WqT.T @ xT          (o = head-major channel)
  kT     [o, kv]   = WkT.T @ xcatT       (kv trimmed to 1101)
  per head h:
    scoresT[kv, n] = kT_h contracted with qT_h over d=64
    E = exp(scoresT / 8)     (ScalarE, psum -> sbuf bf16)
    avp[n,0:65] = sum_kv E[kv,n-tile] * vw[kv, h, :]   (col 64 = S[n]); the
      kv=0 row (gated first key, v0 saved separately) is excluded by zeroing
      vw row 0, the pad rows by trimming the last tile's contraction to 77,
      so E needs no memset cleanup
    attn[n, h*64:+64] = avp[:, :64] / S + tanh(g_h) * v0_h  (one fused
      divide+add DVE op; no reciprocal pass)
  LayerNorm over channels (rows of attn, bf16 input like the reference's
  bf16 cast; ln_g/ln_b pre-folded), then out = LN @ Wp'.T; bp' is added by
  DVE during the PSUM->SBUF copy and the result DMA'd to DRAM.

build_program(reps=K) emits the body K times into one NEFF so the bench
can measure sustained back-to-back execution without per-dispatch overhead.
"""

import os
import numpy as np
import ml_dtypes

import concourse.bacc as bacc
import concourse.tile as tile
from concourse import mybir
from concourse.masks import make_identity
from concourse.bass_utils import run_bass_kernel_spmd

F32 = mybir.dt.float32
BF16 = mybir.dt.bfloat16
AF = mybir.ActivationFunctionType
OP = mybir.AluOpType

B, N, P, DIM, H = 8, 1024, 77, 1024, 16
HD = DIM // H          # 64
KV = P + N             # 1101
KT = 9                 # kv tiles of 128
KVP = KT * 128         # 1152 padded
NT = N // 128          # 8 n tiles
CC = DIM // 128        # 8 contraction chunks
OT = DIM // 128        # 8 output-channel tiles
LN_EPS = 1e-5

LAST_EXEC_NS = None
_CACHE = {}


def _declare_io(nc):
    return {
        "xcatT": nc.dram_tensor("xcatT", [DIM, KVP], BF16, kind="ExternalInput").ap(),
        "wqT": nc.dram_tensor("wqT", [DIM, DIM], BF16, kind="ExternalInput").ap(),
        "wkT": nc.dram_tensor("wkT", [DIM, DIM], BF16, kind="ExternalInput").ap(),
        "wvT": nc.dram_tensor("wvT", [DIM, DIM], BF16, kind="ExternalInput").ap(),
        "wpT": nc.dram_tensor("wpT", [DIM, DIM], BF16, kind="ExternalInput").ap(),
        "tanhg": nc.dram_tensor("tanhg", [1, H], F32, kind="ExternalInput").ap(),
        "bp_bf": nc.dram_tensor("bp_bf", [1, DIM], BF16, kind="ExternalInput").ap(),
        "out": nc.dram_tensor("out", [N, DIM], F32, kind="ExternalOutput").ap(),
    }


def _emit(tc, io):
    nc = tc.nc

    xcatT_d = io["xcatT"]
    wq_d, wk_d, wv_d, wp_d = io["wqT"], io["wkT"], io["wvT"], io["wpT"]
    tanhg_d, bp_d, out_d = io["tanhg"], io["bp_bf"], io["out"]

    xcat_re = xcatT_d.rearrange("(j p) f -> p j f", p=128)
    wq_re = wq_d.rearrange("(j p) o -> p j o", p=128)
    wk_re = wk_d.rearrange("(j p) o -> p j o", p=128)
    wv_re = wv_d.rearrange("(j p) o -> p j o", p=128)
    wp_re = wp_d.rearrange("(j p) o -> p j o", p=128)

    from contextlib import ExitStack

    with ExitStack() as top:
        consts = top.enter_context(tc.tile_pool(name="consts", bufs=1))
        acts = top.enter_context(tc.tile_pool(name="acts", bufs=1))
        # xcatT is double-buffered so the next iteration's input DMAs can
        # land while this iteration still reads the previous buffer; wv/wp
        # get separate buffers (ph1, one per tag) for the same reason.
        # wstream uses 4 bufs so each of the 16 weight tiles per iteration
        # reuses its own slot from the previous iteration (16 % 4 == 0),
        # releasing the WAR hazard as early as possible.
        xpool = top.enter_context(tc.tile_pool(name="xcat", bufs=2))
        ph1 = top.enter_context(tc.tile_pool(name="ph1", bufs=1))
        wstream = top.enter_context(tc.tile_pool(name="wstream", bufs=4))
        qkp = top.enter_context(tc.tile_pool(name="qkp", bufs=3))
        epool = top.enter_context(tc.tile_pool(name="epool", bufs=4))
        tpool = top.enter_context(tc.tile_pool(name="tmp", bufs=4))
        ltp = top.enter_context(tc.tile_pool(name="ltp", bufs=6))
        opool = top.enter_context(tc.tile_pool(name="outp", bufs=2))
        ps_proj = top.enter_context(tc.tile_pool(name="ps_proj", bufs=2, space="PSUM"))
        ps_scores = top.enter_context(
            tc.tile_pool(name="ps_scores", bufs=2, space="PSUM"))
        ps_av = top.enter_context(tc.tile_pool(name="ps_av", bufs=2, space="PSUM"))

        # ---- persistent activations ----
        vw_sb = acts.tile([128, KT, H, HD + 1], BF16, tag="vw")  # [kv-part, kv-tile, h, d+1]
        attn_sb = acts.tile([128, NT, H, HD], BF16, tag="attn")  # [n-part, n-tile, h, d]
        v0row = acts.tile([1, H, HD], BF16, tag="v0row")         # v at kv=0 (gate term)

        # input loads, c-chunk granular; only xcatT is loaded up front --
        # wv/wp loads are emitted later, in consumption order.
        # These are emitted BEFORE the consts ops: the consts memsets wait
        # on the previous iteration's last readers (e.g. ones1's bias
        # matmuls at the very end of the iteration) and would otherwise
        # block the whole pool/act queue -- including these DMA issues --
        # until the iteration boundary. Emitted first, the input DMAs issue
        # mid-iteration as soon as the xcatT/weight regions' readers drain,
        # so the data is already in SBUF when the next iteration starts.
        # They are also split across the gpsimd and scalar DMA rings (the
        # sync ring is still draining the previous iteration's 16 output
        # DMAs; a single ring would serialize ~700ns per DMA).
        xcatT_sb = xpool.tile([128, CC, KVP], BF16, tag="xcatT")
        wv_sb = ph1.tile([128, CC, DIM], BF16, tag="wv")
        w0q = wstream.tile([128, CC, 128], BF16, tag="w")
        nc.gpsimd.dma_start(out=w0q, in_=wq_re[:, :, 0:128])
        for cc in range(CC):
            dmae = nc.scalar if cc % 2 == 0 else nc.gpsimd
            dmae.dma_start(out=xcatT_sb[:, cc, :], in_=xcat_re[:, cc, :])
        w0k = wstream.tile([128, CC, 128], BF16, tag="w")
        nc.scalar.dma_start(out=w0k, in_=wk_re[:, :, 0:128])

        # ---- constants ----
        tanhg_sb = consts.tile([128, H], F32, tag="tanhg")
        nc.sync.dma_start(out=tanhg_sb, in_=tanhg_d.to_broadcast([128, H]))
        bp_sb = consts.tile([1, DIM], BF16, tag="bp")
        nc.sync.dma_start(out=bp_sb, in_=bp_d)
        ones1 = consts.tile([1, 128], BF16, tag="ones1")
        nc.gpsimd.memset(ones1, 1.0)
        eps_t = consts.tile([128, 1], F32, tag="eps")
        nc.vector.memset(eps_t, LN_EPS)
        ident = consts.tile([128, 128], BF16, tag="ident")
        make_identity(nc, ident)

        # ---- q/k projections interleaved with their dependent head pairs,
        # so ScalarE (exp) fills while PE still runs projections ----
        last_rows = KV - (KT - 1) * 128  # 77
        ksplits = [(0, 512), (512, 512), (1024, last_rows)]

        def emit_vproj(kvts):
            # v projection into vw (head-interleaved), natural [kv, o] layout
            for kvt in kvts:
                for half in range(2):
                    ps = ps_proj.tile([128, 512], F32, tag="ps")
                    for cc in range(CC):
                        nc.tensor.matmul(
                            ps,
                            xcatT_sb[:, cc, kvt * 128:(kvt + 1) * 128],
                            wv_sb[:, cc, half * 512:(half + 1) * 512],
                            start=(cc == 0),
                            stop=(cc == CC - 1),
                        )
                    nc.vector.tensor_copy(
                        vw_sb[:, kvt, half * 8:(half + 1) * 8, 0:HD],
                        ps.rearrange("p (h d) -> p h d", d=HD),
                    )

        def emit_qk(ot, wtq=None, wtk=None):
            qt = qkp.tile([128, N], BF16, tag="qt")
            kt = qkp.tile([128, KVP], BF16, tag="kt")
            if wtq is None:
                wtq = wstream.tile([128, CC, 128], BF16, tag="w")
                nc.sync.dma_start(out=wtq, in_=wq_re[:, :, ot * 128:(ot + 1) * 128])
            for half in range(2):
                ps = ps_proj.tile([128, 512], F32, tag="ps")
                for cc in range(CC):
                    nc.tensor.matmul(
                        ps,
                        wtq[:, cc, :],
                        xcatT_sb[:, cc, P + half * 512: P + (half + 1) * 512],
                        start=(cc == 0),
                        stop=(cc == CC - 1),
                    )
                nc.vector.tensor_copy(qt[:, half * 512:(half + 1) * 512], ps)
            if wtk is None:
                wtk = wstream.tile([128, CC, 128], BF16, tag="w")
                nc.sync.dma_start(out=wtk, in_=wk_re[:, :, ot * 128:(ot + 1) * 128])
            for off, width in ksplits:
                ps = ps_proj.tile([128, 512], F32, tag="ps")
                for cc in range(CC):
                    nc.tensor.matmul(
                        ps[:, :width],
                        wtk[:, cc, :],
                        xcatT_sb[:, cc, off:off + width],
                        start=(cc == 0),
                        stop=(cc == CC - 1),
                    )
                nc.vector.tensor_copy(kt[:, off:off + width], ps[:, :width])
            return qt, kt

        def emit_scores_pair(qt, kt):
            # Scores for the even/odd head pair, row-tiled on the PE
            # (K=64 each, partitions 0-63 and 64-127 run back to back).
            ee = epool.tile([128, KT, N], BF16, tag="e")
            eo = epool.tile([128, KT, N], BF16, tag="e")
            for kvt in range(KT):
                rows = last_rows if kvt == KT - 1 else 128
                pse = ps_scores.tile([128, N], F32, tag="pss")
                pso = ps_scores.tile([128, N], F32, tag="pss")
                for half in range(2):
                    nc.tensor.matmul(
                        pse[:rows, half * 512:(half + 1) * 512],
                        kt[0:64, kvt * 128:kvt * 128 + rows],
                        qt[0:64, half * 512:(half + 1) * 512],
                        start=True, stop=True,
                    )
                    nc.tensor.matmul(
                        pso[:rows, half * 512:(half + 1) * 512],
                        kt[64:128, kvt * 128:kvt * 128 + rows],
                        qt[64:128, half * 512:(half + 1) * 512],
                        start=True, stop=True,
                    )
                nc.scalar.activation(
                    ee[:rows, kvt, :], pse[:rows], AF.Exp, bias=0.0, scale=0.125)
                nc.scalar.activation(
                    eo[:rows, kvt, :], pso[:rows], AF.Exp, bias=0.0, scale=0.125)
            return ee, eo

        def prep_gate(h):
            gv0 = tpool.tile([128, HD], BF16, tag="gv0")
            nc.gpsimd.partition_broadcast(gv0, v0row[0:1, h, :])
            gv0s = tpool.tile([128, HD], F32, tag="gv0s")
            nc.vector.tensor_scalar_mul(gv0s, gv0, tanhg_sb[:, h:h + 1])
            return gv0s

        def tail_one(h, e, gv0s, nt, alt=False):
            """AV + fixup for one (head, n-tile). The kv=0 row (gated first
            key) is excluded by zeroing vw row 0 (v0 saved to v0row first);
            the kv pad rows are excluded by trimming the last tile's
            contraction to 77 rows, so E needs no zeroing. alt=True cycles
            the (by-then idle) proj psum pool for deeper AV pipelining."""
            if alt and nt % 2 == 1:
                avp = ps_proj.tile([128, HD + 1], F32, tag="ps")
            else:
                avp = ps_av.tile([128, HD + 1], F32, tag="avp")
            for kvt in range(KT):
                hi = last_rows if kvt == KT - 1 else 128
                nc.tensor.matmul(
                    avp,
                    e[0:hi, kvt, nt * 128:(nt + 1) * 128],
                    vw_sb[0:hi, kvt, h, :],
                    start=(kvt == 0),
                    stop=(kvt == KT - 1),
                )
            # attn = avp * (1/S) + tanh(g)*v0, S in avp col 64 (the DVE
            # ISA has no divide in scalar_tensor_tensor)
            rs = tpool.tile([128, 1], F32, tag="rs")
            nc.vector.reciprocal(rs, avp[:, HD:HD + 1])
            nc.vector.scalar_tensor_tensor(
                out=attn_sb[:, nt, h, :],
                in0=avp[:, 0:HD],
                scalar=rs,
                in1=gv0s,
                op0=OP.mult,
                op1=OP.add,
            )

        def emit_head_tail(h, e, alt=False):
            gv0s = prep_gate(h)
            for nt in range(NT):
                tail_one(h, e, gv0s, nt, alt=alt)

        # software pipeline: scores/exp run one head-pair ahead of the
        # AV/fixup tails so ScalarE never starves
        pend = []
        qt0, kt0 = emit_qk(0, w0q, w0k)
        for cc in range(CC):
            dmae = nc.gpsimd if cc % 2 == 0 else nc.scalar
            dmae.dma_start(out=wv_sb[:, cc, :], in_=wv_re[:, cc, :])
        pend.append(emit_scores_pair(qt0, kt0))
        qt, kt = emit_qk(1)
        pend.append(emit_scores_pair(qt, kt))
        # ones column for the row-sum S; disjoint from the v-projection's
        # columns
        nc.gpsimd.memset(vw_sb[:, :, :, HD:HD + 1], 1.0)
        emit_vproj(range(KT))
        # save v at kv=0 for the gate term, then zero vw row 0 so the AV
        # contraction (and the S ones-column sum) excludes the gated first
        # key without needing E row 0 cleaned up
        nc.gpsimd.tensor_copy(v0row, vw_sb[0:1, 0, :, 0:HD])
        nc.gpsimd.memset(vw_sb[0:1, 0, :, :], 0.0)
        wp_sb = ph1.tile([128, CC, DIM], BF16, tag="wp")
        for cc in range(CC):
            nc.sync.dma_start(out=wp_sb[:, cc, :], in_=wp_re[:, cc, :])
        def emit_ln(nt):
            xa = attn_sb[:, nt].rearrange("p h d -> p (h d)")
            xs = xa.rearrange("p (s f) -> p s f", f=512)
            stats = tpool.tile([128, 2, 6], F32, tag="stats")
            for s in range(2):
                nc.vector.bn_stats(stats[:, s, :], xs[:, s, :])
            mv = tpool.tile([128, 2], F32, tag="mv")
            nc.vector.bn_aggr(mv, stats)
            rstd = tpool.tile([128, 1], F32, tag="rstd")
            nc.scalar.activation(rstd, mv[:, 1:2], AF.Sqrt, bias=eps_t, scale=1.0)
            nc.vector.reciprocal(rstd, rstd)
            # ln_g/ln_b are folded into Wp/bp host-side: L = (x - mu) * rstd
            L_t = qkp.tile([128, DIM], BF16, tag="qt")
            nc.vector.tensor_scalar(
                out=L_t, in0=xa, scalar1=mv[:, 0:1], scalar2=rstd,
                op0=OP.subtract, op1=OP.mult,
            )
            return L_t

        done = 0
        for ot in range(2, OT):
            qt, kt = emit_qk(ot)
            pend.append(emit_scores_pair(qt, kt))
            ep = pend.pop(0)
            emit_head_tail(2 * done, ep[0], alt=(ot >= OT - 2))
            emit_head_tail(2 * done + 1, ep[1], alt=(ot >= OT - 2))
            done += 1
        # second-to-last pair as usual
        ep = pend.pop(0)
        emit_head_tail(2 * done, ep[0], alt=True)
        emit_head_tail(2 * done + 1, ep[1], alt=True)
        done += 1
        # last pair: interleave the two heads per n-tile and emit the early
        # n-tiles' LN chains as soon as both heads' fixups are in, so the
        # PE-side transposes/projection can start right after the last AV
        # instead of idling behind the serial LN dependency chain
        ep = pend.pop(0)
        lns = {}
        ga, gb = prep_gate(2 * done), prep_gate(2 * done + 1)
        for nt in range(NT):
            tail_one(2 * done, ep[0], ga, nt, alt=True)
            tail_one(2 * done + 1, ep[1], gb, nt, alt=True)
            if nt <= 1:
                lns[nt] = emit_ln(nt)
        done += 1

        # ---- LayerNorm + output projection per n-tile; LN emitted one
        # n-tile ahead so the LN chain of nt+1 overlaps the projection of nt ----
        L_t = lns.pop(0)
        for nt in range(NT):
            L_next = lns.pop(nt + 1, None) if nt + 1 < NT else None
            if L_next is None and nt + 1 < NT:
                L_next = emit_ln(nt + 1)

            # transpose LN rows then project: out[n, o] = L @ Wp'.T + bp'
            pp0 = ps_proj.tile([128, 512], F32, tag="ps")
            pp1 = ps_proj.tile([128, 512], F32, tag="ps")
            for cc in range(CC):
                pstp, pstt = (ps_scores, "pss") if cc % 2 == 0 else (ps_av, "avp")
                pst = pstp.tile([128, 128], BF16, tag=pstt)
                nc.tensor.transpose(
                    pst, L_t[:, cc * 128:(cc + 1) * 128], ident
                )
                ltc = ltp.tile([128, 128], BF16, tag="ltc")
                nc.vector.tensor_copy(ltc, pst)
                nc.tensor.matmul(
                    pp0, ltc, wp_sb[:, cc, 0:512],
                    start=(cc == 0), stop=False,
                )
                nc.tensor.matmul(
                    pp1, ltc, wp_sb[:, cc, 512:1024],
                    start=(cc == 0), stop=False,
                )
            # bias as rank-1 accumulation (PSUM is not a legal DMA source,
            # so stage through SBUF)
            nc.tensor.matmul(pp0, ones1, bp_sb[:, 0:512], start=False, stop=True)
            nc.tensor.matmul(pp1, ones1, bp_sb[:, 512:1024], start=False, stop=True)
            ot0 = opool.tile([128, 512], F32, tag="ot")
            nc.scalar.copy(out=ot0, in_=pp0)
            nc.sync.dma_start(out=out_d[nt * 128:(nt + 1) * 128, 0:512], in_=ot0)
            ot1 = opool.tile([128, 512], F32, tag="ot")
            nc.scalar.copy(out=ot1, in_=pp1)
            nc.sync.dma_start(out=out_d[nt * 128:(nt + 1) * 128, 512:1024], in_=ot1)
            L_t = L_next


def build_program(reps=1):
    key = ("nc", reps)
    if key in _CACHE:
        return _CACHE[key]
    nc = bacc.Bacc("TRN2", target_bir_lowering=False, debug=False, num_devices=8)
    with tile.TileContext(nc) as tc:
        io = _declare_io(nc)
        for _ in range(reps):
            _emit(tc, io)
    nc.compile()
    _CACHE[key] = nc
    return nc


def prep_inputs(x, x_text, Wq, Wk, Wv, gate, ln_g, ln_b, Wp, bp):
    """Host-side sharding/layout prep. Returns the 8 per-core input maps."""
    bf = ml_dtypes.bfloat16
    x = np.asarray(x, np.float32)
    x_text = np.asarray(x_text, np.float32)
    xcat = np.concatenate([x_text, x], axis=1)          # [B, KV, DIM]
    xcatT = np.zeros((B, DIM, KVP), np.float32)
    xcatT[:, :, :KV] = xcat.transpose(0, 2, 1)
    xcatT = xcatT.astype(bf)
    wqT = np.ascontiguousarray(np.asarray(Wq, np.float32).T).astype(bf)
    wkT = np.ascontiguousarray(np.asarray(Wk, np.float32).T).astype(bf)
    wvT = np.ascontiguousarray(np.asarray(Wv, np.float32).T).astype(bf)
    # fold LayerNorm affine into the output projection:
    #   (L*g + b) @ Wp.T + bp == L @ (Wp*g).T + (bp + Wp @ b)
    Wp = np.asarray(Wp, np.float32)
    g = np.asarray(ln_g, np.float32).reshape(DIM)
    bvec = np.asarray(ln_b, np.float32).reshape(DIM)
    Wpf = Wp * g[None, :]
    bpf = np.asarray(bp, np.float32).reshape(DIM) + Wp @ bvec
    wpT = np.ascontiguousarray(Wpf.T).astype(bf)
    tanhg = np.tanh(np.asarray(gate, np.float32)).reshape(1, H).astype(np.float32)
    bp_bf = bpf.reshape(1, DIM).astype(bf)
    in_maps = []
    for b in range(B):
        in_maps.append({
            "xcatT": np.ascontiguousarray(xcatT[b]),
            "wqT": wqT, "wkT": wkT, "wvT": wvT, "wpT": wpT,
            "tanhg": tanhg, "bp_bf": bp_bf,
        })
    return in_maps


def kernel(**inputs):
    global LAST_EXEC_NS
    nc = build_program()
    in_maps = prep_inputs(**inputs)
    trace = bool(int(os.environ.get("BASS_TRACE_RUN", "0")))
    res = run_bass_kernel_spmd(
        nc, in_maps, core_ids=list(range(8)), trace=trace,
    )
    LAST_EXEC_NS = res.exec_time_ns
    out = np.stack([r["out"] for r in res.results], axis=0)
    return out.astype(np.float32)
